# revision 7
# baseline (speedup 1.0000x reference)
"""Trainium2 Bass kernel for nn_DynamicReceptiveEncoder.

Reference computation (per batch element):
  x [W=512 time, F=25] -> conv3x3 & conv7x7 (1->64 ch, SAME) over (F, W)
  -> temporal |diff| of each -> four LIF neuron scans over W -> sum of spikes
  -> out [C=64, F=25, W=512].

Sharding: data-parallel over batch, B=32 -> 8 cores x 4.

Per-core dataflow (all engines overlapped, scheduled by Tile):
  PE   : one fp32 matmul pass computes conv3 & conv7 together (K=59 patch
         rows incl. bias row, M=128 = both conv's channels), plus bf16
         matmuls that sum spikes over the 4 neurons.
  DMA  : im2col staging of conv patches straight from DRAM, output store.
  ACT  : conv PSUM->SBUF eviction, |diff|*1.25 (theta-normalize), out evict.
  GPSIMD: temporal diff (a_t - a_{t-1}).
  DVE  : the sequential 512-step LIF scan (2 fused mult-adds + 1 fused
         compare-mask-mult per step) and batched spike compares.
"""

import sys

sys.path.insert(0, "/opt/trn_rl_repo")

import numpy as np

import concourse.bass as bass
import concourse.mybir as mybir
from concourse.tile import TileContext
from concourse import bass_utils

AL = mybir.AluOpType
AF = mybir.ActivationFunctionType
F32 = mybir.dt.float32
BF16 = mybir.dt.bfloat16

# ---------------------------------------------------------------------------
# Patches for this walrus build (max ONE sync wait per instruction) and for
# the missing NTFF profile hook module.
# ---------------------------------------------------------------------------
import concourse.tile as _tile
from concourse.vector_clock import ScopedClock as _ScopedClock

_wsplit_counter = [0]


def _patched_drain_and_barrier(self, tick_clock, wait_clock):
    nc = self.nc
    drain_inst = nc.sync.drain()
    wait_clock.add_sem_waits(
        drain_inst.ins, _ScopedClock({None: tick_clock.global_clock})
    )
    si = drain_inst.ins.sync_info
    waits = list(si.on_wait) if si is not None else []
    if len(waits) > 1:
        updates = list(si.on_update) if si is not None else []
        drain_inst.ins.sync_info = mybir.SyncInfo(on_wait=[], on_update=updates)
        for w in waits:
            nop_inst = nc.sync.nop(nofuse=True)
            nop_inst.ins.sync_info = mybir.SyncInfo(on_wait=[w], on_update=[])

    nc.all_engine_barrier()
    assert self.sems is not None
    popped = nc._tile_sem_poison_stack.pop()
    assert popped is self._sem_poison
    nc.clear_and_free_semaphores(list(self.sems.allocated().values()))
    nc.all_engine_barrier()


_tile.TileContext._drain_and_barrier = _patched_drain_and_barrier


def _split_multi_waits(nc, max_waits=1):
    for f in nc.m.functions:
        for bb in f.blocks:
            insts = bb.instructions
            i = 0
            while i < len(insts):
                inst = insts[i]
                si = inst.sync_info
                if si is not None and len(si.on_wait) > max_waits:
                    waits = list(si.on_wait)
                    extra, keep = waits[:-max_waits], waits[-max_waits:]
                    inst.sync_info = mybir.SyncInfo(
                        on_wait=keep, on_update=list(si.on_update)
                    )
                    for w in extra:
                        _wsplit_counter[0] += 1
                        nop = mybir.InstNoOp(
                            name=f"wsplit_{_wsplit_counter[0]}", ins=[], outs=[]
                        )
                        nop.engine = inst.engine
                        nop.sync_info = mybir.SyncInfo(on_wait=[w], on_update=[])
                        insts.insert(i, nop)
                        i += 1
                i += 1


def _install_ntff_hook():
    import contextlib, ctypes, types

    try:
        lib = ctypes.CDLL("/opt/axon/libaxon_pjrt.so")
    except OSError:
        return
    if not hasattr(lib, "axon_start_nrt_profile"):
        return
    lib.axon_start_nrt_profile.argtypes = [
        ctypes.POINTER(ctypes.c_int64),
        ctypes.c_size_t,
    ]
    lib.axon_start_nrt_profile.restype = ctypes.c_int64
    lib.axon_stop_nrt_profile.argtypes = [ctypes.c_char_p]
    lib.axon_stop_nrt_profile.restype = ctypes.c_int64

    @contextlib.contextmanager
    def _hook(output_dir, device_ids):
        import jax

        jax.devices()
        if device_ids:
            ids = (ctypes.c_int64 * len(device_ids))(*device_ids)
            rc = lib.axon_start_nrt_profile(ids, len(device_ids))
        else:
            rc = lib.axon_start_nrt_profile(None, 0)
        if rc != 0:
            raise RuntimeError(f"axon_start_nrt_profile rc={rc}")
        try:
            yield
        finally:
            lib.axon_stop_nrt_profile(str(output_dir).encode())

    mod = types.ModuleType("antenv.axon_hooks")
    holder = [_hook]
    mod.set_axon_ntff_profile_hook = lambda h: holder.__setitem__(0, h)
    mod.get_axon_ntff_profile_hook = lambda: holder[0]
    sys.modules["antenv.axon_hooks"] = mod
    try:
        import antenv

        antenv.axon_hooks = mod
    except ImportError:
        pass


_install_ntff_hook()

# ---------------------------------------------------------------------------
# Problem constants (hardcoded from the spec)
# ---------------------------------------------------------------------------
B, W, F, C = 32, 512, 25, 64
NCORES = 8
BL = B // NCORES            # 4 batch elements per core
FP, WP = F + 6, W + 6       # padded field: [31, 518]
NBF = BL * F                # 100 (b, f) columns per time step
K = 49                      # 7x7 patch rows; 3x3 shares them, bias via ACT

T_RHS = 32                  # time steps per staged im2col chunk
T_XA = 32                   # time steps per conv-output SBUF chunk
T_DP = 16                   # time steps per temporal-diff chunk
T_SUB = 4                   # time steps per PSUM matmul (400 cols <= bank)
T_V = 8                     # time steps per v/s chunk (spike batch)
T_OUT = 128                 # time steps per output DMA chunk

TAU = (20.0, 50.0, 2.0, 0.91)
VTH = (1.0, 1.0, 0.8, 0.8)
ALPHA = tuple(1.0 - 1.0 / t for t in TAU)   # python float64, cast later
BSCALE = 1.0 / VTH[2]       # 1.25 exactly; normalizes the d-side threshold


def _build_nc():
    nc = bass.Bass()
    xpad = nc.dram_tensor("xpad", [FP, BL, WP], F32, kind="ExternalInput")
    wcat = nc.dram_tensor("wcat", [K, 128], F32, kind="ExternalInput")
    wsum = nc.dram_tensor("wsum", [128, C], BF16, kind="ExternalInput")
    biasv = nc.dram_tensor("biasv", [128, 1], F32, kind="ExternalInput")
    alphaA = nc.dram_tensor("alphaA", [128, 1], F32, kind="ExternalInput")
    alphaB = nc.dram_tensor("alphaB", [128, 1], F32, kind="ExternalInput")
    outp = nc.dram_tensor("out", [BL, C, F, W], BF16, kind="ExternalOutput")

    xpad_flat = xpad.rearrange("f b w -> (f b w)")
    FSTR = BL * WP            # stride of one padded-f row

    def patch_row_ap(t0, i, pj, base):
        # one kernel row i: partitions = j shifts, free = (fb merged, t)
        return bass.AP(
            tensor=xpad_flat.tensor,
            offset=base + i * FSTR + t0,
            ap=[[1, pj], [WP, NBF], [1, T_RHS]],
        )

    with TileContext(nc) as tc:
        with (
            tc.tile_pool(name="consts", bufs=1) as cpool,
            tc.tile_pool(name="rhs", bufs=3) as rhspool,
            tc.tile_pool(name="xa", bufs=3) as xapool,
            tc.tile_pool(name="dp", bufs=2) as dppool,
            tc.tile_pool(name="xb", bufs=3) as xbpool,
            tc.tile_pool(name="vchunk", bufs=2) as vpool,
            tc.tile_pool(name="schunk", bufs=2) as spool,
            tc.tile_pool(name="state", bufs=1) as wpool,
            tc.tile_pool(name="outsb", bufs=2) as opool,
            tc.tile_pool(name="sab", bufs=2) as sabpool,
            tc.tile_pool(name="psA", bufs=4, space="PSUM") as psA,
            tc.tile_pool(name="psO", bufs=4, space="PSUM") as psO,
        ):
            wcat_sb = cpool.tile([K, 128], F32, name="wcat_sb")
            nc.sync.dma_start(wcat_sb[:], wcat[:])
            wsum_sb = cpool.tile([128, C], BF16, name="wsum_sb")
            nc.sync.dma_start(wsum_sb[:], wsum[:])
            aA = cpool.tile([128, 1], F32, name="aA")
            nc.sync.dma_start(aA[:], alphaA[:])
            aB = cpool.tile([128, 1], F32, name="aB")
            nc.sync.dma_start(aB[:], alphaB[:])
            bias_sb = cpool.tile([128, 1], F32, name="bias_sb")
            nc.sync.dma_start(bias_sb[:], biasv[:])

            # LIF state: cols 0:100 = A-side (a3|a7), 100:200 = B-side
            wst = wpool.tile([128, 2 * NBF], F32, name="wst")
            nc.vector.memset(wst[:], 0.0)

            xa_tiles = {}   # chunk index -> tile (conv out, (bf, t) layout)
            xb_tiles = {}
            v_tiles = {}
            s_tiles = {}
            out_tiles = {}

            for ci in range(W // T_XA):
                t0 = ci * T_XA
                rhs = rhspool.tile([K, NBF * T_RHS], F32, name="rhs")
                rhs_w = rhs.ap[0][0]

                def rhs_rows(p0, pj):
                    return bass.AP(
                        tensor=rhs.tensor,
                        offset=rhs.offset + p0 * rhs_w,
                        ap=[[rhs_w, pj], [T_RHS, NBF], [1, T_RHS]],
                    )

                for i in range(7):
                    nc.sync.dma_start(
                        rhs_rows(i * 7, 7),
                        patch_row_ap(t0, i, 7, 0),
                    )

                # conv matmuls; xa chunk is (t, bf) contiguous
                xa = xapool.tile([128, NBF * T_XA], F32, name="xa")
                xa_tiles[ci] = xa
                for sub in range(T_XA // T_SUB):
                    ts = sub * T_SUB
                    pa = psA.tile([128, NBF * T_SUB], F32, name="pa")
                    rhs_slice = bass.AP(
                        tensor=rhs.tensor,
                        offset=rhs.offset + ts,
                        ap=[[rhs_w, K], [1, T_SUB], [T_RHS, NBF]],
                    )
                    nc.tensor.matmul(
                        pa[:].rearrange("p (t bf) -> p t bf", t=T_SUB),
                        wcat_sb[:],
                        rhs_slice,
                        start=True,
                        stop=True,
                    )
                    nc.scalar.activation(
                        xa[:, ts * NBF : (ts + T_SUB) * NBF],
                        pa[:],
                        AF.Identity,
                        bias=bias_sb[:],
                        scale=1.0,
                    )

                # temporal diff + |.|*1.25 for the B side, T_DP chunks
                for dsub in range(T_XA // T_DP):
                    di = ci * (T_XA // T_DP) + dsub
                    td = dsub * T_DP
                    dp = dppool.tile([128, NBF * T_DP], F32, name="dp")
                    nc.gpsimd.tensor_tensor(
                        out=dp[:, NBF : T_DP * NBF],
                        in0=xa[:, (td + 1) * NBF : (td + T_DP) * NBF],
                        in1=xa[:, td * NBF : (td + T_DP - 1) * NBF],
                        op=AL.subtract,
                    )
                    if di == 0:
                        nc.gpsimd.memset(dp[:, 0:NBF], 0.0)
                    else:
                        if td == 0:
                            prev = xa_tiles[ci - 1]
                            pin = prev[:, (T_XA - 1) * NBF : T_XA * NBF]
                        else:
                            pin = xa[:, (td - 1) * NBF : td * NBF]
                        nc.gpsimd.tensor_tensor(
                            out=dp[:, 0:NBF],
                            in0=xa[:, td * NBF : (td + 1) * NBF],
                            in1=pin,
                            op=AL.subtract,
                        )
                    xb = xbpool.tile([128, NBF * T_DP], F32, name="xb")
                    xb_tiles[di] = xb
                    nc.scalar.activation(
                        xb[:], dp[:], AF.Abs, bias=0.0, scale=BSCALE
                    )

                # ---- the sequential LIF scan for these T_XA steps ----
                for tl in range(T_XA):
                    t = t0 + tl
                    vi = t // T_V
                    if t % T_V == 0:
                        v_tiles[vi] = vpool.tile(
                            [128, 2 * NBF * T_V], F32, name="vch"
                        )
                    v = v_tiles[vi]
                    vo = (t % T_V) * 2 * NBF
                    xbt = xb_tiles[t // T_DP]
                    # v_A = alphaA*w_A + x_A[t]
                    nc.vector.scalar_tensor_tensor(
                        out=v[:, vo : vo + NBF],
                        in0=wst[:, 0:NBF],
                        scalar=aA[:],
                        in1=xa[:, tl * NBF : (tl + 1) * NBF],
                        op0=AL.mult,
                        op1=AL.add,
                    )
                    # v_B = alphaB*w_B + x_B[t]
                    nc.vector.scalar_tensor_tensor(
                        out=v[:, vo + NBF : vo + 2 * NBF],
                        in0=wst[:, NBF : 2 * NBF],
                        scalar=aB[:],
                        in1=xbt[:, (t % T_DP) * NBF : (t % T_DP + 1) * NBF],
                        op0=AL.mult,
                        op1=AL.add,
                    )
                    # w = (v < 1) * v   (hard reset)
                    nc.vector.scalar_tensor_tensor(
                        out=wst[:],
                        in0=v[:, vo : vo + 2 * NBF],
                        scalar=1.0,
                        in1=v[:, vo : vo + 2 * NBF],
                        op0=AL.is_lt,
                        op1=AL.mult,
                    )

                    if t % T_V == T_V - 1:
                        # batched spike compare for the whole chunk
                        s = spool.tile([128, 2 * NBF * T_V], BF16, name="sch")
                        s_tiles[vi] = s
                        nc.vector.tensor_scalar(
                            out=s[:],
                            in0=v[:],
                            scalar1=1.0,
                            scalar2=0.0,
                            op0=AL.is_ge,
                        )
                        s_w = s.ap[0][0]
                        # fold A+B halves on GPSIMD (bf16 adds, exact)
                        sab = sabpool.tile([128, NBF * T_V], BF16, name="sab")
                        nc.gpsimd.tensor_tensor(
                            out=sab[:],
                            in0=bass.AP(
                                tensor=s.tensor,
                                offset=s.offset,
                                ap=[[s_w, 128], [2 * NBF, T_V], [1, NBF]],
                            ),
                            in1=bass.AP(
                                tensor=s.tensor,
                                offset=s.offset + NBF,
                                ap=[[s_w, 128], [2 * NBF, T_V], [1, NBF]],
                            ),
                            op=AL.add,
                        )
                        sab_w = sab.ap[0][0]
                        for g in range(T_V // T_SUB):
                            tg = vi * T_V + g * T_SUB
                            po = psO.tile([C, NBF * T_SUB], F32, name="po")
                            rhs_s = bass.AP(
                                tensor=sab.tensor,
                                offset=sab.offset + g * T_SUB * NBF,
                                ap=[[sab_w, 128], [1, NBF], [NBF, T_SUB]],
                            )
                            nc.tensor.matmul(
                                po[:].rearrange("p (bf t) -> p bf t", t=T_SUB),
                                wsum_sb[:],
                                rhs_s,
                                start=True,
                                stop=True,
                            )
                            oi = tg // T_OUT
                            if oi not in out_tiles:
                                out_tiles[oi] = opool.tile(
                                    [C, NBF * T_OUT], BF16, name="och"
                                )
                            ot = out_tiles[oi]
                            ot_w = ot.ap[0][0]
                            dst = bass.AP(
                                tensor=ot.tensor,
                                offset=ot.offset + (tg % T_OUT),
                                ap=[
                                    [ot_w, C],
                                    [T_OUT, NBF],
                                    [1, T_SUB],
                                ],
                            )
                            nc.scalar.activation(
                                dst,
                                po[:].rearrange("p (bf t) -> p bf t", t=T_SUB),
                                AF.Copy,
                            )
                            if tg % T_OUT == T_OUT - T_SUB:
                                # chunk complete -> DMA to DRAM (per b)
                                tbase = (tg // T_OUT) * T_OUT
                                for bb in range(BL):
                                    srcap = bass.AP(
                                        tensor=ot.tensor,
                                        offset=ot.offset + bb * T_OUT,
                                        ap=[
                                            [ot_w, C],
                                            [BL * T_OUT, F],
                                            [1, T_OUT],
                                        ],
                                    )
                                    dstap = bass.AP(
                                        tensor=outp[:].tensor,
                                        offset=bb * C * F * W + tbase,
                                        ap=[
                                            [F * W, C],
                                            [W, F],
                                            [1, T_OUT],
                                        ],
                                    )
                                    nc.gpsimd.dma_start(dstap, srcap)

    _split_multi_waits(nc)
    return nc


_NC_CACHE = [None]
LAST_RESULT = [None]


def _get_nc():
    if _NC_CACHE[0] is None:
        _NC_CACHE[0] = _build_nc()
    return _NC_CACHE[0]


def _prep_inputs(inputs, w3, b3, w7, b7):
    """Host-side (cheap) prep: pad/transpose input, assemble weights."""
    f32 = np.float32
    # weights: [K, 128]; cols 0:64 conv3 channels, 64:128 conv7 channels
    wcat = np.zeros((K, 128), dtype=f32)
    w3r = w3.reshape(C, 3, 3)
    for i in range(3):
        for j in range(3):
            wcat[(i + 2) * 7 + (j + 2), 0:C] = w3r[:, i, j]
    wcat[0:49, C:128] = w7.reshape(C, 49).T
    biasv = np.concatenate([b3, b7]).astype(f32).reshape(128, 1)

    import ml_dtypes

    wsum = np.zeros((128, C), dtype=np.float32)
    wsum[np.arange(C), np.arange(C)] = 1.0
    wsum[np.arange(C) + C, np.arange(C)] = 1.0
    wsum = wsum.astype(ml_dtypes.bfloat16)

    alphaA = np.full((128, 1), f32(ALPHA[0]), dtype=f32)
    alphaA[64:, 0] = f32(ALPHA[1])
    alphaB = np.full((128, 1), f32(ALPHA[2]), dtype=f32)
    alphaB[64:, 0] = f32(ALPHA[3])

    per_core = []
    for ci in range(NCORES):
        xb = inputs[ci * BL : (ci + 1) * BL]          # [4, 512, 25]
        xpad = np.zeros((FP, BL, WP), dtype=f32)
        xpad[3 : 3 + F, :, 3 : 3 + W] = np.transpose(xb, (2, 0, 1))
        per_core.append(
            {
                "xpad": np.ascontiguousarray(xpad),
                "wcat": wcat,
                "wsum": wsum,
                "biasv": biasv,
                "alphaA": alphaA,
                "alphaB": alphaB,
            }
        )
    return per_core


def kernel(inputs, w3, b3, w7, b7):
    nc = _get_nc()
    per_core = _prep_inputs(
        np.asarray(inputs, dtype=np.float32),
        np.asarray(w3, dtype=np.float32),
        np.asarray(b3, dtype=np.float32),
        np.asarray(w7, dtype=np.float32),
        np.asarray(b7, dtype=np.float32),
    )
    res = bass_utils.run_bass_kernel_spmd(
        nc, per_core, core_ids=list(range(NCORES))
    )
    LAST_RESULT[0] = res
    out = np.concatenate(
        [np.asarray(res.results[i]["out"]).astype(np.float32) for i in range(NCORES)],
        axis=0,
    )
    return out
